# revision 1
# baseline (speedup 1.0000x reference)
"""Trainium2 Bass kernel for nn_MixAttention (GAT-style mixed attention).

Strategy (8 cores, i-sharded over query rows, transposed compute):
  - Device computes scores in transposed layout [j on partitions, i free] so
    out^T += hc_chunk.T @ P^T_chunk contracts over partitions, no transposes.
  - Host passes h_context.T / h_structure.T (layout prep) and param-folded
    projection vectors; real FLOPs (projections, softmax, scores, attention)
    stay on device.
  - Mask passed as complement-uint8 in a partition-major layout (long DMA
    lines); fused on DVE as (-L*maskC + bcB) so exp underflows masked
    entries to exactly 0 (identical math to the -9e15 additive mask).
  - exp(alpha - M0) with a host-precomputed upper bound M0 (numerical shim
    only; cancels exactly in the rowsum division).
  - rowsum via M=1 ones-stationary matmul sharing the P^T moving operand.
  - Engine balance: ACT does preluB + half of preluA + exp; DVE does the
    mask-fuse STT + the other half of preluA (tensor_scalar + max trick) +
    1/3 of the adds; GPSIMD does 2/3 of the adds; PE uses fp32r (TF32-like)
    for the large matmuls (element rounding ~5e-4, cancels partially in the
    softmax normalization; measured end-to-end rel err ~2e-4).
  - Phase 0 (projections/grids) is streamed in 1MB groups and fully
    pipelined with phase 1; mask slab DMAs interleave on the sync queue.
"""

import numpy as np

N = 8192
K = 256
F = 128
NC = 8
S = N // NC  # 1024 rows per core
NEG_L = 1.0e6
GRP = 2      # j-chunks per exp/matmul group

_BUILD_CACHE = {}


def _build_program(cA, cB):
    import contextlib

    import concourse.bacc as bacc
    import concourse.tile as tile
    from concourse import mybir

    nc = bacc.Bacc("TRN2", target_bir_lowering=False, debug=False, num_devices=NC)
    dt = mybir.dt
    AF = mybir.ActivationFunctionType
    OP = mybir.AluOpType

    hctxT = nc.dram_tensor("hctxT", [K, N], dt.float32, kind="ExternalInput")
    hstrT = nc.dram_tensor("hstrT", [K, N], dt.float32, kind="ExternalInput")
    hctxT_my = nc.dram_tensor("hctxT_my", [K, S], dt.float32, kind="ExternalInput")
    hstrT_my = nc.dram_tensor("hstrT_my", [K, S], dt.float32, kind="ExternalInput")
    wvA = nc.dram_tensor("wvA", [K, F + 1], dt.float32, kind="ExternalInput")
    vA = nc.dram_tensor("vA", [K, 2], dt.float32, kind="ExternalInput")
    uB = nc.dram_tensor("uB", [K, 3], dt.float32, kind="ExternalInput")
    maskP = nc.dram_tensor("maskP", [128, (N // 128) * S], dt.uint8,
                           kind="ExternalInput")
    negm0 = nc.dram_tensor("negm0", [128, 1], dt.float32, kind="ExternalInput")
    outT = nc.dram_tensor("outT", [F, S], dt.float32, kind="ExternalOutput")

    NCH = N // 128   # 64 j-chunks
    KC = K // 128    # 2 contraction chunks
    NSLAB = 16
    SLABC = NCH // NSLAB
    G0 = 8           # j-chunks per phase-0 stream group
    W0 = G0 * 128

    with tile.TileContext(nc) as tc:
        with contextlib.ExitStack() as ctx:
            vecs = ctx.enter_context(tc.tile_pool(name="vecs", bufs=1))
            hcpool = ctx.enter_context(tc.tile_pool(name="hc", bufs=1))
            stp = ctx.enter_context(tc.tile_pool(name="stream", bufs=2))
            work = ctx.enter_context(tc.tile_pool(name="work", bufs=3))
            grpp = ctx.enter_context(tc.tile_pool(name="grp", bufs=3))
            slabp = ctx.enter_context(tc.tile_pool(name="slabp", bufs=2))

            # ---- small inputs ----
            vA_sb = [vecs.tile([128, 2], dt.float32, name=f"vA{k}") for k in range(KC)]
            uB_sb = [vecs.tile([128, 3], dt.float32, name=f"uB{k}") for k in range(KC)]
            wvA_sb = [vecs.tile([128, F + 1], dt.float32, name=f"wvA{k}")
                      for k in range(KC)]
            negm0_sb = vecs.tile([128, 1], dt.float32, name="negm0_sb")
            nc.sync.dma_start(negm0_sb[:], negm0.ap())
            my_str = [stp.tile([128, S], dt.float32, name=f"mystr{k}", tag=f"hst{k}", bufs=3)
                      for k in range(KC)]
            my_ctx = [stp.tile([128, S], dt.float32, name=f"myctx{k}", tag=f"hct{k}", bufs=3)
                      for k in range(KC)]
            for k in range(KC):
                ks = slice(128 * k, 128 * (k + 1))
                nc.sync.dma_start(vA_sb[k][:], vA.ap()[ks, :])
                nc.sync.dma_start(uB_sb[k][:], uB.ap()[ks, :])
                nc.sync.dma_start(wvA_sb[k][:], wvA.ap()[ks, :])
                nc.sync.dma_start(my_str[k][:], hstrT_my.ap()[ks, :])
                nc.sync.dma_start(my_ctx[k][:], hctxT_my.ap()[ks, :])
            for k in range(KC):
                nc.scalar.activation(my_str[k][:], my_str[k][:], AF.Exp)

            # ---- src rows for my i-slice ----
            sigrow = work.tile([1, S], dt.float32, name="sigrow", tag="u")
            srcBraw = work.tile([1, S], dt.float32, name="srcBraw", tag="tA")
            srcArow = work.tile([1, S], dt.float32, name="srcArow", tag="tB")
            with tc.tile_pool(name="psrow", bufs=1, space="PSUM") as psrow:
                psr0 = psrow.tile([1, S], dt.float32, name="psr0")
                psr1 = psrow.tile([1, S], dt.float32, name="psr1")
                psra = psrow.tile([1, S], dt.float32, name="psra")
                for k in range(KC):
                    st, sp = (k == 0), (k == KC - 1)
                    for h in range(S // 512):
                        hs_ = slice(512 * h, 512 * (h + 1))
                        nc.tensor.matmul(psr0[:, hs_], uB_sb[k][:, 0:1],
                                         my_str[k][:, hs_], start=st, stop=sp)
                        nc.tensor.matmul(psr1[:, hs_], uB_sb[k][:, 2:3],
                                         my_str[k][:, hs_], start=st, stop=sp)
                        nc.tensor.matmul(psra[:, hs_], vA_sb[k][:, 0:1],
                                         my_ctx[k][:, hs_], start=st, stop=sp)
                nc.vector.tensor_copy(sigrow[:], psr0[:])
                nc.vector.tensor_copy(srcBraw[:], psr1[:])
                if cA != 0.0:
                    nc.vector.tensor_scalar_add(srcArow[:], psra[:], cA)
                else:
                    nc.vector.tensor_copy(srcArow[:], psra[:])

            srecrow = work.tile([1, S], dt.float32, name="srecrow", tag="u")
            srcBrow = work.tile([1, S], dt.float32, name="srcBrow", tag="tA")
            nc.vector.reciprocal(srecrow[:], sigrow[:])
            nc.vector.tensor_tensor(srcBrow[:], srcBraw[:], srecrow[:], OP.mult)
            if cB != 0.0:
                nc.vector.tensor_scalar_add(srcBrow[:], srcBrow[:], cB)

            ones_row = vecs.tile([1, 128], dt.float32, name="ones_row")
            nc.vector.memset(ones_row[:], 1.0)
            ones_colf = vecs.tile([128, 1], dt.float32, name="ones_colf")
            nc.vector.memset(ones_colf[:], 1.0)
            ones_col = vecs.tile([128, 1], dt.float32r, name="ones_col")
            nc.vector.tensor_copy(ones_col[:], ones_colf[:])

            bcA = vecs.tile([128, S], dt.float32, name="bcA")
            bcB = vecs.tile([128, S], dt.float32, name="bcB")
            with tc.tile_pool(name="ps0c", bufs=1, space="PSUM") as ps0c:
                psbc = ps0c.tile([128, S], dt.float32, name="psbc")
                psbc2 = ps0c.tile([128, S], dt.float32, name="psbc2")
                for h in range(S // 512):
                    hs_ = slice(512 * h, 512 * (h + 1))
                    nc.tensor.matmul(psbc[:, hs_], ones_row[:], srcArow[:, hs_],
                                     start=True, stop=True)
                    nc.tensor.matmul(psbc2[:, hs_], ones_row[:], srcBrow[:, hs_],
                                     start=True, stop=True)
                nc.vector.tensor_copy(bcA[:], psbc[:])
                nc.vector.tensor_copy(bcB[:], psbc2[:])

            # ---- grids + hc, streamed; slab DMAs interleaved ----
            sgrid = vecs.tile([128, NCH], dt.float32, name="sgrid")
            bgrid = vecs.tile([128, NCH], dt.float32, name="bgrid")
            agrid = vecs.tile([128, NCH], dt.float32, name="agrid")
            hc_sb = [hcpool.tile([128, F], dt.float32r, name=f"hc{c}")
                     for c in range(NCH)]
            slabs = []
            with tc.tile_pool(name="ps0", bufs=2, space="PSUM") as ps0:
                for g in range(NCH // G0):
                    for t in (2 * g, 2 * g + 1):
                        slab = slabp.tile([128, SLABC * S], dt.uint8, name="slab",
                                          bufs=3)
                        nc.sync.dma_start(
                            slab[:],
                            maskP.ap()[:, t * SLABC * S:(t + 1) * SLABC * S])
                        slabs.append(slab)
                    gs = slice(W0 * g, W0 * (g + 1))
                    hst = [stp.tile([128, W0], dt.float32, name=f"hstg{k}",
                                    tag=f"hst{k}", bufs=3) for k in range(KC)]
                    hct = [stp.tile([128, W0], dt.float32, name=f"hctg{k}",
                                    tag=f"hct{k}", bufs=3) for k in range(KC)]
                    for k in range(KC):
                        ks = slice(128 * k, 128 * (k + 1))
                        nc.sync.dma_start(hst[k][:], hstrT.ap()[ks, gs])
                        nc.gpsimd.dma_start(hct[k][:], hctxT.ap()[ks, gs])
                        nc.scalar.activation(hst[k][:], hst[k][:], AF.Exp)
                    for cc in range(G0):
                        c = G0 * g + cc
                        cs = slice(128 * cc, 128 * (cc + 1))
                        psb = ps0.tile([128, 2], dt.float32, name="psb")
                        psA = ps0.tile([128, F + 1], dt.float32, name="psA")
                        for k in range(KC):
                            st, sp = (k == 0), (k == KC - 1)
                            nc.tensor.matmul(psb[:], hst[k][:, cs],
                                             uB_sb[k][:, 0:2], start=st, stop=sp)
                            nc.tensor.matmul(psA[:], hct[k][:, cs],
                                             wvA_sb[k][:], start=st, stop=sp)
                        nc.vector.reciprocal(sgrid[:, c:c + 1], psb[:, 0:1])
                        nc.vector.tensor_tensor(bgrid[:, c:c + 1], psb[:, 1:2],
                                                sgrid[:, c:c + 1], OP.mult)
                        nc.vector.tensor_copy(hc_sb[c][:], psA[:, 0:F])
                        if cA != 0.0:
                            nc.vector.tensor_scalar_add(agrid[:, c:c + 1],
                                                        psA[:, F:F + 1], cA)
                        else:
                            nc.vector.tensor_copy(agrid[:, c:c + 1],
                                                  psA[:, F:F + 1])
                if cB != 0.0:
                    nc.vector.tensor_scalar_add(bgrid[:], bgrid[:], cB)

            # ---- phase 1, pipelined with the stream loop above ----
            with tc.tile_pool(name="ps1", bufs=1, space="PSUM") as ps1:
                outT_ps = ps1.tile([F, S], dt.float32, name="outT_ps")
                rs_ps = ps1.tile([1, S], dt.float32, name="rs_ps")
                for t in range(NSLAB):
                    slab = slabs[t]
                    for g in range(SLABC // GRP):
                        sgrp = grpp.tile([128, GRP * S], dt.float32,
                                         name="sgrp")
                        Pgrp = grpp.tile([128, GRP * S], dt.float32r,
                                         name="Pgrp")
                        for cc in range(GRP):
                            c = t * SLABC + g * GRP + cc
                            lo = (g * GRP + cc) * S
                            o = cc * S
                            u = work.tile([128, S], dt.float32, name="u")
                            nc.vector.scalar_tensor_tensor(
                                u[:], slab[:, lo:lo + S], -NEG_L, bcB[:],
                                OP.mult, OP.add)
                            tB = work.tile([128, S], dt.float32, name="tB")
                            nc.scalar.activation(tB[:], u[:], AF.Prelu,
                                                 bias=bgrid[:, c:c + 1],
                                                 scale=1.0, alpha=0.01)
                            tA = work.tile([128, S], dt.float32, name="tA")
                            if c % 2 == 1:
                                sA = work.tile([128, S], dt.float32, name="sAt")
                                nc.vector.tensor_scalar(
                                    sA[:], bcA[:], agrid[:, c:c + 1], None,
                                    OP.add)
                                nc.vector.scalar_tensor_tensor(
                                    tA[:], sA[:], 0.01, sA[:], OP.mult, OP.max)
                            else:
                                nc.scalar.activation(tA[:], bcA[:], AF.Prelu,
                                                     bias=agrid[:, c:c + 1],
                                                     scale=1.0, alpha=0.01)
                            if c % 3 == 0:
                                nc.vector.tensor_tensor(
                                    sgrp[:, o:o + S], tA[:], tB[:], OP.add)
                            else:
                                nc.gpsimd.tensor_tensor(
                                    sgrp[:, o:o + S], tA[:], tB[:], OP.add)
                        nc.scalar.activation(Pgrp[:], sgrp[:], AF.Exp,
                                             bias=negm0_sb[:], scale=1.0)
                        c0 = t * SLABC + g * GRP
                        st = (c0 == 0)
                        sp = (c0 + GRP == NCH)
                        for cc in range(GRP):
                            c = c0 + cc
                            for h in range(S // 512):
                                hs_ = slice(cc * S + 512 * h,
                                            cc * S + 512 * (h + 1))
                                ps_ = slice(512 * h, 512 * (h + 1))
                                nc.tensor.matmul(outT_ps[:, ps_],
                                                 hc_sb[c][:], Pgrp[:, hs_],
                                                 start=st and cc == 0,
                                                 stop=sp and cc == GRP - 1)
                                nc.tensor.matmul(rs_ps[:, ps_],
                                                 ones_col[:], Pgrp[:, hs_],
                                                 start=st and cc == 0,
                                                 stop=sp and cc == GRP - 1)

                # normalize and write out
                rs_sb = work.tile([1, S], dt.float32, name="rs_sb", tag="tB")
                nc.vector.tensor_scalar_add(rs_sb[:], rs_ps[:], 1e-30)
                rrec = work.tile([1, S], dt.float32, name="rrec", tag="sAt")
                nc.vector.reciprocal_approx_fast(rrec[:], rs_sb[:])
                rbc_ps = ps1.tile([128, S], dt.float32, name="rbc_ps")
                for h in range(S // 512):
                    hs_ = slice(512 * h, 512 * (h + 1))
                    nc.tensor.matmul(rbc_ps[:, hs_], ones_row[:],
                                     rrec[:, hs_], start=True, stop=True)
                rbc = work.tile([128, S], dt.float32, name="rbcs", tag="u")
                nc.vector.tensor_copy(rbc[:], rbc_ps[:])
                out_sb = work.tile([F, S], dt.float32, name="out_sb", tag="tA")
                nc.vector.tensor_tensor(out_sb[:], outT_ps[:], rbc[:],
                                        OP.mult)
                nc.sync.dma_start(outT.ap(), out_sb[:])

    nc.compile()
    return nc


def kernel(h_context, h_structure, edge_index, Wc_w, Wc_b, Ws_w, Ws_b,
           ac_w, as_w, Ws_coff, Wc_coff):
    from concourse.bass_utils import run_bass_kernel_spmd

    h_context = np.asarray(h_context, np.float32)
    h_structure = np.asarray(h_structure, np.float32)
    Wc_w = np.asarray(Wc_w, np.float32)
    Wc_b = np.asarray(Wc_b, np.float32)
    Ws_w = np.asarray(Ws_w, np.float32)
    Ws_b = np.asarray(Ws_b, np.float32)
    ac_w = np.asarray(ac_w, np.float32)
    as_w = np.asarray(as_w, np.float32)
    ei = np.asarray(edge_index)

    wA = float(abs(np.float32(np.asarray(Ws_coff)[0, 0])))  # scales alpha_c
    wB = float(abs(np.float32(np.asarray(Wc_coff)[0, 0])))  # scales alpha_s

    vA_np = np.stack([Wc_w.T @ ac_w[0, :F], Wc_w.T @ ac_w[0, F:]], axis=1) * wA
    uB_np = np.stack([
        np.ones(K, np.float32),
        wB * (Ws_w.T @ as_w[0, F:]),   # dstB proj
        wB * (Ws_w.T @ as_w[0, :F]),   # srcB proj
    ], axis=1).astype(np.float32)
    cA = wA * float(Wc_b @ ac_w[0, :F] + Wc_b @ ac_w[0, F:])
    cB = wB * float(Ws_b @ as_w[0, :F] + Ws_b @ as_w[0, F:])

    key = (round(cA, 12), round(cB, 12))
    if key not in _BUILD_CACHE:
        _BUILD_CACHE[key] = _build_program(cA, cB)
    nc = _BUILD_CACHE[key]

    # complement adjacency, transposed + partition-major re-layout
    maskCT = np.ones((N, N), np.uint8)
    maskCT[ei[1], ei[0]] = 0

    hctxT = np.ascontiguousarray(h_context.T)
    hstrT = np.ascontiguousarray(h_structure.T)
    vA_np = np.ascontiguousarray(vA_np.astype(np.float32))
    wvA_np = np.ascontiguousarray(
        np.concatenate([Wc_w.T, vA_np[:, 1:2]], axis=1).astype(np.float32))
    uB_np = np.ascontiguousarray(uB_np)

    # host M0 shim: upper bound of alpha per core (cancels in division)
    lrelu = lambda x: np.where(x > 0, x, 0.01 * x)
    srcA = h_context @ (vA_np[:, 0]) + cA          # wA folded
    dstA = h_context @ (vA_np[:, 1])
    e_str = np.exp(h_structure - h_structure.max(axis=1, keepdims=True))
    sm = e_str / e_str.sum(axis=1, keepdims=True)
    srcB = sm @ uB_np[:, 2] + cB
    dstB = sm @ uB_np[:, 1]
    dstA_max = float(dstA.max())
    dstB_max = float(dstB.max())

    in_maps = []
    for d in range(NC):
        sl = slice(S * d, S * (d + 1))
        m0_d = (lrelu(float(srcA[sl].max()) + dstA_max)
                + lrelu(float(srcB[sl].max()) + dstB_max))
        maskP = np.ascontiguousarray(
            maskCT[:, sl].reshape(N // 128, 128, S)
            .transpose(1, 0, 2).reshape(128, (N // 128) * S))
        in_maps.append({
            "hctxT": hctxT,
            "hstrT": hstrT,
            "hctxT_my": np.ascontiguousarray(hctxT[:, sl]),
            "hstrT_my": np.ascontiguousarray(hstrT[:, sl]),
            "wvA": wvA_np,
            "vA": vA_np,
            "uB": uB_np,
            "maskP": maskP,
            "negm0": np.full((128, 1), -np.float32(m0_d), np.float32),
        })

    res = run_bass_kernel_spmd(nc, in_maps, core_ids=list(range(NC)))
    out = np.empty((N, F), np.float32)
    for d in range(NC):
        out[S * d:S * (d + 1), :] = res.results[d]["outT"].T

    # rows with no edges: reference gives uniform attention = mean of hc
    row_deg = np.zeros(N, np.int64)
    np.add.at(row_deg, ei[0], 1)
    empty = row_deg == 0
    if empty.any():
        hc_host = h_context @ Wc_w.T + Wc_b
        out[empty, :] = hc_host.mean(axis=0)

    return out

